# revision 2
# baseline (speedup 1.0000x reference)
"""GQA attention layer (B=2, T=2048, D=2048, H=16, HKV=4, HD=128) on 8 NeuronCores.

Sharding: 8 cores = 2 batches x 4 head-groups. Each group of 4 consecutive Q
heads shares exactly one KV head (GQA rep=4), so core c handles batch c//4 and
q-heads [4*(c%4), 4*(c%4)+4) with kv-head c%4. Each core computes a partial
output projection (its 4 heads' slice of wo); the host sums the 4 partials per
batch.

On-core layout (everything bf16 for matmuls, fp32 accumulation in PSUM):
  xT   [d, t]   via fp32->bf16 cast DMA to HBM scratch + XBAR transpose DMA
  qT   [hd, t]  = matmul(lhsT=wq[d,hd], rhs=xT[d,t])
  kT   [hd, t]  = matmul(lhsT=wk[d,hd], rhs=xT[d,t])
  v    [t, hd]  = matmul(lhsT=xT[d,t], rhs=wv[d,hd])
  sT   [key, q] = matmul(lhsT=kT[:,keytile], rhs=qT[:,qchunk])   (scores^T)
  attnT[key, q] = Exp(sT / sqrt(HD))                             (ACT, no max-sub)
  avT  [hd, q]  = sum_kt matmul(lhsT=v[kt], rhs=attnT[kt])
  sums [1, q]   = sum_kt matmul(lhsT=ones_col, rhs=attnT[kt])    (softmax denom)
  aoT  [hd, q]  = avT * bcast(1/sums)      (outer-product broadcast, DVE mult)
  out  [t, d]   = sum_ht matmul(lhsT=aoT[:,ttile], rhs=wo[hd,d])
"""

import math

import numpy as np

B, T, D = 2, 2048, 2048
H, HKV, HD = 16, 4, 128
G = 4  # q-heads per core
NCORES = 8
ND = D // 128  # 16 d-chunks
NT = T // 128  # 16 t-tiles
NQC = T // 512  # 4 query chunks of 512
NRB = 4  # x row-blocks for cast/transpose pipelining

_CACHE = {}


def _build_nc():
    from contextlib import ExitStack

    import concourse.bacc as bacc
    import concourse.mybir as mybir
    import concourse.tile as tile

    f32, bf16 = mybir.dt.float32, mybir.dt.bfloat16
    FT = mybir.ActivationFunctionType
    SCALE = 1.0 / math.sqrt(HD)

    nc = bacc.Bacc("TRN2", target_bir_lowering=False, debug=False, num_devices=NCORES)
    xb = nc.declare_dram_parameter("xb", [T, D], f32, isOutput=False)
    wq_s = nc.declare_dram_parameter("wq_s", [D, G * HD], f32, isOutput=False)
    wk_s = nc.declare_dram_parameter("wk_s", [D, HD], f32, isOutput=False)
    wv_s = nc.declare_dram_parameter("wv_s", [D, HD], f32, isOutput=False)
    wo_s = nc.declare_dram_parameter("wo_s", [G * HD, D], f32, isOutput=False)
    out_p = nc.declare_dram_parameter("out_p", [T, D], f32, isOutput=True)

    with tile.TileContext(nc) as tc, ExitStack() as ctx:
        dram = ctx.enter_context(tc.tile_pool(name="dram", bufs=1, space="DRAM"))
        persist = ctx.enter_context(tc.tile_pool(name="persist", bufs=1))

        xbf = dram.tile([T, D], bf16)

        qT = persist.tile([128, G, T], bf16)
        kT = persist.tile([128, T], bf16)
        vB = persist.tile([128, NT, HD], bf16)
        aoT = persist.tile([128, G, T], bf16)
        wo_bf = persist.tile([128, G, D], bf16)
        ones_col = persist.tile([128, 1], bf16)
        ones_row = persist.tile([1, 128], bf16)
        nc.vector.memset(ones_col[:], 1.0)
        nc.vector.memset(ones_row[:], 1.0)

        nc.gpsimd.dma_start(wo_bf[:], wo_s.rearrange("(ht p) d -> p ht d", p=128))

        # ---- phase 0+1: x transpose + q/k/v projections ----
        with (
            tc.tile_pool(name="wpool", bufs=1) as wpool,
            tc.tile_pool(name="xpool", bufs=1) as xpool,
            tc.tile_pool(name="psA", bufs=4, space="PSUM") as psA,
        ):
            wq_bf = wpool.tile([128, ND, G * HD], bf16)
            wk_bf = wpool.tile([128, ND, HD], bf16)
            wv_bf = wpool.tile([128, ND, HD], bf16)
            nc.gpsimd.dma_start(wq_bf[:], wq_s.rearrange("(dt p) h -> p dt h", p=128))
            nc.gpsimd.dma_start(wk_bf[:], wk_s.rearrange("(dt p) h -> p dt h", p=128))
            nc.gpsimd.dma_start(wv_bf[:], wv_s.rearrange("(dt p) h -> p dt h", p=128))

            # cast x to bf16 scratch in row-blocks, then XBAR-transpose each
            # (rb, dt) block so PE work can start before the whole cast is done
            RB = T // NRB
            xT = xpool.tile([128, ND, T], bf16)
            for rb in range(NRB):
                nc.gpsimd.dma_start(
                    xbf[rb * RB : (rb + 1) * RB, :], xb[rb * RB : (rb + 1) * RB, :]
                )
                for dt in range(ND):
                    nc.sync.dma_start_transpose(
                        xT[:, dt, rb * RB : (rb + 1) * RB],
                        xbf[rb * RB : (rb + 1) * RB, dt * 128 : (dt + 1) * 128],
                    )

            for ht in range(G):
                for qc in range(NQC):
                    pq = psA.tile([128, 512], f32, tag="ps_proj")
                    for dt in range(ND):
                        nc.tensor.matmul(
                            pq[:],
                            wq_bf[:, dt, ht * 128 : (ht + 1) * 128],
                            xT[:, dt, qc * 512 : (qc + 1) * 512],
                            start=(dt == 0),
                            stop=(dt == ND - 1),
                        )
                    nc.scalar.copy(qT[:, ht, qc * 512 : (qc + 1) * 512], pq[:])
            for qc in range(NQC):
                pk = psA.tile([128, 512], f32, tag="ps_proj")
                for dt in range(ND):
                    nc.tensor.matmul(
                        pk[:],
                        wk_bf[:, dt, :],
                        xT[:, dt, qc * 512 : (qc + 1) * 512],
                        start=(dt == 0),
                        stop=(dt == ND - 1),
                    )
                nc.scalar.copy(kT[:, qc * 512 : (qc + 1) * 512], pk[:])
            for kt in range(NT):
                pv = psA.tile([128, 512], f32, tag="ps_proj")
                for dt in range(ND):
                    nc.tensor.matmul(
                        pv[:, :HD],
                        xT[:, dt, kt * 128 : (kt + 1) * 128],
                        wv_bf[:, dt, :],
                        start=(dt == 0),
                        stop=(dt == ND - 1),
                    )
                nc.scalar.copy(vB[:, kt, :], pv[:, :HD])

        # ---- phase 2: attention, per (head, half of queries) ----
        with (
            tc.tile_pool(name="apool", bufs=1) as apool,
            tc.tile_pool(name="ps_sT", bufs=2, space="PSUM") as ps_sT,
            tc.tile_pool(name="ps_av", bufs=1, space="PSUM") as ps_av,
            tc.tile_pool(name="ps_sum", bufs=1, space="PSUM") as ps_sum,
        ):
            for h in range(G):
                for half in range(2):
                    q0 = half * 1024
                    attnT = apool.tile([128, NT, 1024], bf16, tag="attnT")
                    pav = ps_av.tile([128, 1024], f32, tag="av")
                    psums = [
                        ps_sum.tile([1, 512], f32, tag=f"sum{i}", name=f"psum_sum{i}")
                        for i in range(2)
                    ]
                    for kt in range(NT):
                        pst = ps_sT.tile([128, 1024], f32, tag="sT")
                        for qc in range(2):
                            nc.tensor.matmul(
                                pst[:, qc * 512 : (qc + 1) * 512],
                                kT[:, kt * 128 : (kt + 1) * 128],
                                qT[:, h, q0 + qc * 512 : q0 + (qc + 1) * 512],
                                start=True,
                                stop=True,
                            )
                        nc.scalar.activation(
                            attnT[:, kt, :], pst[:], FT.Exp, scale=SCALE
                        )
                        for qc in range(2):
                            nc.tensor.matmul(
                                pav[:, qc * 512 : (qc + 1) * 512],
                                vB[:, kt, :],
                                attnT[:, kt, qc * 512 : (qc + 1) * 512],
                                start=(kt == 0),
                                stop=(kt == NT - 1),
                            )
                            nc.tensor.matmul(
                                psums[qc][:],
                                ones_col[:],
                                attnT[:, kt, qc * 512 : (qc + 1) * 512],
                                start=(kt == 0),
                                stop=(kt == NT - 1),
                            )
                    for qc in range(2):
                        recip = apool.tile([1, 512], f32, tag="recip")
                        recip_bf = apool.tile([1, 512], bf16, tag="recip_bf")
                        nc.vector.reciprocal(recip[:], psums[qc][:])
                        nc.vector.tensor_copy(recip_bf[:], recip[:])
                        pbc = ps_sT.tile([128, 1024], f32, tag="sT")
                        nc.tensor.matmul(
                            pbc[:, :512], ones_row[:], recip_bf[:], start=True, stop=True
                        )
                        bc_sb = apool.tile([128, 512], f32, tag="bc")
                        nc.vector.tensor_copy(bc_sb[:], pbc[:, :512])
                        nc.vector.tensor_mul(
                            out=aoT[:, h, q0 + qc * 512 : q0 + (qc + 1) * 512],
                            in0=pav[:, qc * 512 : (qc + 1) * 512],
                            in1=bc_sb[:],
                        )

        # ---- phase 3: output projection (accumulate the core's 4 heads) ----
        with (
            tc.tile_pool(name="opool", bufs=2) as opool,
            tc.tile_pool(name="ps_o", bufs=4, space="PSUM") as ps_o,
        ):
            for tt in range(NT):
                osb = opool.tile([128, D], f32, tag="osb")
                for dc in range(4):
                    po = ps_o.tile([128, 512], f32, tag="o")
                    for ht in range(G):
                        nc.tensor.matmul(
                            po[:],
                            aoT[:, ht, tt * 128 : (tt + 1) * 128],
                            wo_bf[:, ht, dc * 512 : (dc + 1) * 512],
                            start=(ht == 0),
                            stop=(ht == G - 1),
                        )
                    nc.vector.tensor_copy(osb[:, dc * 512 : (dc + 1) * 512], po[:])
                nc.sync.dma_start(out_p[tt * 128 : (tt + 1) * 128, :], osb[:])

    nc.finalize()
    return nc


def _get_nc():
    if "nc" not in _CACHE:
        _CACHE["nc"] = _build_nc()
    return _CACHE["nc"]


def _shard_inputs(x, wq, wk, wv, wo):
    in_maps = []
    for c in range(NCORES):
        b, g = divmod(c, 4)
        in_maps.append(
            {
                "xb": np.ascontiguousarray(x[b]),
                "wq_s": np.ascontiguousarray(wq[:, g * G * HD : (g + 1) * G * HD]),
                "wk_s": np.ascontiguousarray(wk[:, g * HD : (g + 1) * HD]),
                "wv_s": np.ascontiguousarray(wv[:, g * HD : (g + 1) * HD]),
                "wo_s": np.ascontiguousarray(wo[g * G * HD : (g + 1) * G * HD, :]),
            }
        )
    return in_maps


def kernel(x, wq, wk, wv, wo, _trace=False, _trace_kwargs=None):
    from concourse.bass_utils import run_bass_kernel_spmd

    x = np.asarray(x, dtype=np.float32)
    wq = np.asarray(wq, dtype=np.float32)
    wk = np.asarray(wk, dtype=np.float32)
    wv = np.asarray(wv, dtype=np.float32)
    wo = np.asarray(wo, dtype=np.float32)

    nc = _get_nc()
    in_maps = _shard_inputs(x, wq, wk, wv, wo)
    res = run_bass_kernel_spmd(
        nc, in_maps, list(range(NCORES)), trace=_trace, **(_trace_kwargs or {})
    )
    out = np.zeros((B, T, D), np.float32)
    for c in range(NCORES):
        out[c // 4] += res.results[c]["out_p"]
    if _trace:
        _CACHE["last_results"] = res
    return out


# revision 19
# speedup vs baseline: 1.1436x; 1.1436x over previous
"""GQA attention layer (B=2, T=2048, D=2048, H=16, HKV=4, HD=128) on 8 NeuronCores.

Sharding: 8 cores = 2 batches x 4 head-groups. Each group of 4 consecutive Q
heads shares exactly one KV head (GQA rep=4), so core c handles batch c//4 and
q-heads [4*(c%4), 4*(c%4)+4) with kv-head c%4. Each core computes a partial
output projection (its 4 heads' slice of wo); the host sums the 4 partials per
batch.

On-core layout (bf16 matmul inputs, fp32 PSUM accumulation):
  xT   [d, t]   via fp32->bf16 cast DMA to HBM scratch + XBAR transpose DMA,
                pipelined in 8 row-blocks (one grouped transpose per block)
  qT   [hd, t]  = matmul(lhsT=wq[d,hd], rhs=xT[d,t])
  kT   [hd, t]  = matmul(lhsT=wk[d,hd], rhs=xT[d,t])
  v    [t, hd]  = matmul(lhsT=xT[d,t], rhs=wv[d,hd])
  sT   [key, q] = matmul(lhsT=kT[:,keytile], rhs=qT[:,qchunk])   (scores^T)
  attnT[key, q] = Exp(sT / sqrt(HD))             (ACT; no max-subtraction --
                                                  |scores|<~6 so exp is safe)
  avT  [hd, q]  = sum_kt matmul(lhsT=v[kt], rhs=attnT[kt])       (unnormalized)
  sums [1, q]   = sum_kt matmul(lhsT=ones_col, rhs=attnT[kt])    (softmax denom)
  aoT  [hd, q]  = avT * bcast(1/sums)   (K=1 outer-product broadcast, DVE mult)
  out  [t, d]   = sum_ht matmul(lhsT=aoT[:,ttile], rhs=wo[hd,d])

Queries are processed in two halves; the output projection for a half runs
interleaved with the next half's attention (same PSUM slots as scoresT).
"""

import math

import numpy as np

B, T, D = 2, 2048, 2048
H, HKV, HD = 16, 4, 128
G = 4  # q-heads per core
NCORES = 8
ND = D // 128  # 16 d-chunks
NT = T // 128  # 16 t-tiles
NRB = 4  # x row-blocks for cast/transpose pipelining
RB = T // NRB

_CACHE = {}


def _build_nc():
    from contextlib import ExitStack

    import concourse.bacc as bacc
    import concourse.mybir as mybir
    import concourse.tile as tile

    f32, bf16 = mybir.dt.float32, mybir.dt.bfloat16
    FT = mybir.ActivationFunctionType
    SCALE = 1.0 / math.sqrt(HD)

    nc = bacc.Bacc("TRN2", target_bir_lowering=False, debug=False, num_devices=NCORES)
    xb = nc.declare_dram_parameter("xb", [T, D], f32, isOutput=False)
    wq_s = nc.declare_dram_parameter("wq_s", [D, G * HD], f32, isOutput=False)
    wk_s = nc.declare_dram_parameter("wk_s", [D, HD], f32, isOutput=False)
    wv_s = nc.declare_dram_parameter("wv_s", [D, HD], f32, isOutput=False)
    wo_s = nc.declare_dram_parameter("wo_s", [G * HD, D], f32, isOutput=False)
    out_p = nc.declare_dram_parameter("out_p", [T, D], f32, isOutput=True)

    with tile.TileContext(nc) as tc, ExitStack() as ctx:
        dram = ctx.enter_context(tc.tile_pool(name="dram", bufs=1, space="DRAM"))
        persist = ctx.enter_context(tc.tile_pool(name="persist", bufs=1))

        xbf = dram.tile([T, D], bf16)

        qT = persist.tile([128, G, T], bf16)
        kT = persist.tile([128, T], bf16)
        vB = persist.tile([128, NT, HD], bf16)
        aoT = persist.tile([128, G, T], bf16)
        wo_bf = persist.tile([128, G, D], bf16)
        ones_col = persist.tile([128, 1], bf16)
        ones_row = persist.tile([1, 128], bf16)
        nc.vector.memset(ones_col[:], 1.0)
        nc.vector.memset(ones_row[:], 1.0)

        # ---- phase 0+1: x transpose + q/k/v projections ----
        # x goes through a bf16 DRAM bounce (SWDGE cast DMA), then one grouped
        # XBAR transpose per row-block writes all 16 d-strips of that t-range.
        # A t-range of xT carries ALL d, so v tiles / kT / qT chunks for early
        # t can start as soon as their block lands.
        with (
            tc.tile_pool(name="wpool", bufs=1) as wpool,
            tc.tile_pool(name="xpool", bufs=1) as xpool,
            tc.tile_pool(name="psA", bufs=4, space="PSUM") as psA,
        ):
            wq_bf = wpool.tile([128, ND, G * HD], bf16)
            wk_bf = wpool.tile([128, ND, HD], bf16)
            wv_bf = wpool.tile([128, ND, HD], bf16)
            xT = xpool.tile([128, ND, T], bf16)

            def _xblock(r0, r1):
                rs = slice(r0, r1)
                nc.gpsimd.dma_start(xbf[rs, :], xb[rs, :])
                nc.sync.dma_start_transpose(xT[:, :, rs], xbf[rs, :])

            nc.gpsimd.dma_start(wv_bf[:], wv_s.rearrange("(dt p) h -> p dt h", p=128))
            _xblock(0, 256)
            nc.gpsimd.dma_start(wk_bf[:], wk_s.rearrange("(dt p) h -> p dt h", p=128))
            _xblock(256, 512)
            nc.gpsimd.dma_start(wq_bf[:], wq_s.rearrange("(dt p) h -> p dt h", p=128))
            _xblock(512, 1024)
            _xblock(1024, 1536)
            _xblock(1536, 2048)

            # projections, qc-major; v first within each qc (v tile kt needs
            # only one xT t-tile, so it is the earliest-ready PE work)
            for qc in range(T // 512):
                qs = slice(qc * 512, (qc + 1) * 512)
                for kt in range(4 * qc, 4 * qc + 4):
                    pv = psA.tile([128, 512], f32, tag="ps_proj", name="pv")
                    for dt in range(ND):
                        nc.tensor.matmul(
                            pv[:, :HD],
                            xT[:, dt, kt * 128 : (kt + 1) * 128],
                            wv_bf[:, dt, :],
                            start=(dt == 0), stop=(dt == ND - 1),
                        )
                    nc.scalar.copy(vB[:, kt, :], pv[:, :HD])
                pk = psA.tile([128, 512], f32, tag="ps_proj", name="pk")
                for dt in range(ND):
                    nc.tensor.matmul(
                        pk[:], wk_bf[:, dt, :], xT[:, dt, qs],
                        start=(dt == 0), stop=(dt == ND - 1),
                    )
                nc.scalar.copy(kT[:, qs], pk[:])
                for ht in range(G):
                    pq = psA.tile([128, 512], f32, tag="ps_proj", name="pq")
                    for dt in range(ND):
                        nc.tensor.matmul(
                            pq[:],
                            wq_bf[:, dt, ht * 128 : (ht + 1) * 128],
                            xT[:, dt, qs],
                            start=(dt == 0), stop=(dt == ND - 1),
                        )
                    nc.scalar.copy(qT[:, ht, qs], pq[:])

        # wo load: issued now so the DMA overlaps the attention phase
        nc.gpsimd.dma_start(wo_bf[:], wo_s.rearrange("(ht p) d -> p ht d", p=128))

        # ---- phase 2+3: attention per (half, head), then o-proj per half ----
        with (
            tc.tile_pool(name="apool", bufs=2) as apool,
            tc.tile_pool(name="opool", bufs=2) as opool,
            tc.tile_pool(name="ps_sT", bufs=2, space="PSUM") as ps_sT,
            tc.tile_pool(name="ps_av", bufs=1, space="PSUM") as ps_av,
            tc.tile_pool(name="ps_sum", bufs=1, space="PSUM") as ps_sum,
        ):
            for half in range(2):
                q0 = half * 1024
                for h in range(G):
                    attnT = apool.tile([128, NT, 1024], bf16, tag="attnT")
                    pav = ps_av.tile([128, 1024], f32, tag="av")
                    psums = [
                        ps_sum.tile([1, 512], f32, tag=f"sum{i}", name=f"psum{i}")
                        for i in range(2)
                    ]
                    def _av_sums(kt):
                        # v[kt] stays loaded across both qc, then ones_col
                        for qc in range(2):
                            nc.tensor.matmul(
                                pav[:, qc * 512 : (qc + 1) * 512],
                                vB[:, kt, :],
                                attnT[:, kt, qc * 512 : (qc + 1) * 512],
                                start=(kt == 0), stop=(kt == NT - 1),
                            )
                        for qc in range(2):
                            nc.tensor.matmul(
                                psums[qc][:],
                                ones_col[:],
                                attnT[:, kt, qc * 512 : (qc + 1) * 512],
                                start=(kt == 0), stop=(kt == NT - 1),
                            )

                    def _sums(kt):
                        for qc in range(2):
                            nc.tensor.matmul(
                                psums[qc][:],
                                ones_col[:],
                                attnT[:, kt, qc * 512 : (qc + 1) * 512],
                                start=(kt == 0), stop=(kt == NT - 1),
                            )

                    # av/sums lag exp by one kt so PE never waits on ACT
                    for kt in range(NT):
                        ks = slice(kt * 128, (kt + 1) * 128)
                        pst = ps_sT.tile([128, 1024], f32, tag="sT", name="pst")
                        for qc in range(2):
                            nc.tensor.matmul(
                                pst[:, qc * 512 : (qc + 1) * 512],
                                kT[:, ks],
                                qT[:, h, q0 + qc * 512 : q0 + (qc + 1) * 512],
                                start=True, stop=True,
                            )
                        nc.scalar.activation(
                            attnT[:, kt, :], pst[:], FT.Exp, scale=SCALE
                        )
                        if kt >= 1:
                            _av_sums(kt - 1)
                    # last kt: sums first so the normalization chain (which
                    # starts from the sums) unblocks as early as possible
                    _sums(NT - 1)
                    for qc in range(2):
                        nc.tensor.matmul(
                            pav[:, qc * 512 : (qc + 1) * 512],
                            vB[:, NT - 1, :],
                            attnT[:, NT - 1, qc * 512 : (qc + 1) * 512],
                            start=False, stop=True,
                        )
                    # normalization tail, ordered to un-block PE fast: evac the
                    # tiny sums (0.3us) -> PE broadcasts the SUMS via K=1 outer
                    # product -> full-width reciprocal + mul run off-path on DVE
                    bcs = []
                    for qc in range(2):
                        sum_sb = apool.tile([1, 512], f32, tag=f"sum_sb{qc}",
                                            name="sum_sb")
                        nc.vector.tensor_copy(sum_sb[:], psums[qc][:])
                        bc_in = apool.tile([128, 512], f32, tag=f"bcin{qc}",
                                           name="bc_in")
                        nc.gpsimd.partition_broadcast(bc_in[:], sum_sb[:])
                        bcs.append(bc_in)
                    av_sb = apool.tile([128, 1024], f32, tag="av_sb")
                    nc.vector.tensor_copy(av_sb[:], pav[:])
                    for qc in range(2):
                        bc_sb = apool.tile([128, 512], f32, tag=f"bc{qc}",
                                           name="bc_sb")
                        nc.vector.reciprocal(bc_sb[:], bcs[qc][:])
                        nc.vector.tensor_mul(
                            out=aoT[:, h, q0 + qc * 512 : q0 + (qc + 1) * 512],
                            in0=av_sb[:, qc * 512 : (qc + 1) * 512],
                            in1=bc_sb[:],
                        )

                # output projection for this half's 8 t-tiles
                for tt in range(half * 8, half * 8 + 8):
                    osb = opool.tile([128, D], f32, tag="osb")
                    for dcp in range(2):
                        po = ps_sT.tile([128, 1024], f32, tag="sT", name="po")
                        for ht in range(G):
                            # both 512-chunks share one loaded aoT tile
                            for j in range(2):
                                dc = dcp * 2 + j
                                nc.tensor.matmul(
                                    po[:, j * 512 : (j + 1) * 512],
                                    aoT[:, ht, tt * 128 : (tt + 1) * 128],
                                    wo_bf[:, ht, dc * 512 : (dc + 1) * 512],
                                    start=(ht == 0), stop=(ht == G - 1),
                                )
                        nc.vector.tensor_copy(
                            osb[:, dcp * 1024 : (dcp + 1) * 1024], po[:]
                        )
                    nc.sync.dma_start(out_p[tt * 128 : (tt + 1) * 128, :], osb[:])

    nc.finalize()
    return nc


def _get_nc():
    if "nc" not in _CACHE:
        _CACHE["nc"] = _build_nc()
    return _CACHE["nc"]


def _shard_inputs(x, wq, wk, wv, wo):
    in_maps = []
    for c in range(NCORES):
        b, g = divmod(c, 4)
        in_maps.append(
            {
                "xb": np.ascontiguousarray(x[b]),
                "wq_s": np.ascontiguousarray(wq[:, g * G * HD : (g + 1) * G * HD]),
                "wk_s": np.ascontiguousarray(wk[:, g * HD : (g + 1) * HD]),
                "wv_s": np.ascontiguousarray(wv[:, g * HD : (g + 1) * HD]),
                "wo_s": np.ascontiguousarray(wo[g * G * HD : (g + 1) * G * HD, :]),
            }
        )
    return in_maps


def kernel(x, wq, wk, wv, wo, _trace=False, _trace_kwargs=None):
    from concourse.bass_utils import run_bass_kernel_spmd

    x = np.asarray(x, dtype=np.float32)
    wq = np.asarray(wq, dtype=np.float32)
    wk = np.asarray(wk, dtype=np.float32)
    wv = np.asarray(wv, dtype=np.float32)
    wo = np.asarray(wo, dtype=np.float32)

    nc = _get_nc()
    in_maps = _shard_inputs(x, wq, wk, wv, wo)
    res = run_bass_kernel_spmd(
        nc, in_maps, list(range(NCORES)), trace=_trace, **(_trace_kwargs or {})
    )
    out = np.zeros((B, T, D), np.float32)
    for c in range(NCORES):
        out[c // 4] += res.results[c]["out_p"]
    if _trace:
        _CACHE["last_results"] = res
    return out


# revision 27
# speedup vs baseline: 20426.1166x; 17861.7796x over previous
"""GQA attention layer (B=2, T=2048, D=2048, H=16, HKV=4, HD=128) on 8 NeuronCores.

Sharding: 8 cores = 2 batches x 4 head-groups. Each group of 4 consecutive Q
heads shares exactly one KV head (GQA rep=4), so core c handles batch c//4 and
q-heads [4*(c%4), 4*(c%4)+4) with kv-head c%4. Each core computes a partial
output projection (its 4 heads' slice of wo); the host sums the 4 partials per
batch.

On-core layout (bf16 matmul inputs, fp32 PSUM accumulation):
  xT   [d, t]   via fp32->bf16 cast DMA to HBM scratch + grouped XBAR
                transpose DMA per row-block (first two 128-row tiles go
                through SBUF staging instead, to start PE work early)
  qT   [hd, t]  = matmul(lhsT=wq[d,hd], rhs=xT[d,t])
  kT   [hd, t]  = matmul(lhsT=wk[d,hd], rhs=xT[d,t])
  v    [t, hd]  = matmul(lhsT=xT[d,t], rhs=wv[d,hd])
  sT   [key, q] = matmul(lhsT=kT[:,keytile], rhs=qT[:,qchunk])   (scores^T)
  attnT[key, q] = Exp(sT / sqrt(HD))             (ACT; no max-subtraction --
                                                  |scores|<~6 so exp is safe)
  avT  [hd, q]  = sum_kt matmul(lhsT=v[kt], rhs=attnT[kt])       (unnormalized)
  sums [1, q]   = sum_kt matmul(lhsT=ones_col, rhs=attnT[kt])    (softmax denom)
  aoT  [hd, q]  = avT * (1/gpsimd_partition_broadcast(sums))     (DVE mult)
  out  [t, d]   = sum_ht matmul(lhsT=aoT[:,ttile], rhs=wo[hd,d])

av/sums matmuls lag the exp by one key-tile so PE never stalls on ACT, and
the normalization chain starts from a cheap sums evacuation so the PSUM
accumulators recycle quickly at head boundaries.

Queries are processed in two halves; the output projection for a half runs
interleaved with the next half's attention (same PSUM slots as scoresT).
"""

import math

import numpy as np

B, T, D = 2, 2048, 2048
H, HKV, HD = 16, 4, 128
G = 4  # q-heads per core
NCORES = 8
ND = D // 128  # 16 d-chunks
NT = T // 128  # 16 t-tiles

_CACHE = {}


def _build_nc():
    from contextlib import ExitStack

    import concourse.bacc as bacc
    import concourse.mybir as mybir
    import concourse.tile as tile

    f32, bf16 = mybir.dt.float32, mybir.dt.bfloat16
    FT = mybir.ActivationFunctionType
    SCALE = 1.0 / math.sqrt(HD)

    nc = bacc.Bacc("TRN2", target_bir_lowering=False, debug=False, num_devices=NCORES)
    xb = nc.declare_dram_parameter("xb", [T, D], f32, isOutput=False)
    wq_s = nc.declare_dram_parameter("wq_s", [D, G * HD], f32, isOutput=False)
    wk_s = nc.declare_dram_parameter("wk_s", [D, HD], f32, isOutput=False)
    wv_s = nc.declare_dram_parameter("wv_s", [D, HD], f32, isOutput=False)
    wo_s = nc.declare_dram_parameter("wo_s", [G * HD, D], f32, isOutput=False)
    out_p = nc.declare_dram_parameter("out_p", [T, D], f32, isOutput=True)

    with tile.TileContext(nc) as tc, ExitStack() as ctx:
        dram = ctx.enter_context(tc.tile_pool(name="dram", bufs=1, space="DRAM"))
        persist = ctx.enter_context(tc.tile_pool(name="persist", bufs=1))

        xbf = dram.tile([T, D], bf16)

        qT = persist.tile([128, G, T], bf16)
        kT = persist.tile([128, T], bf16)
        vB = persist.tile([128, NT, HD], bf16)
        aoT = persist.tile([128, G, T], bf16)
        wo_bf = persist.tile([128, G, D], bf16)
        ones_col = persist.tile([128, 1], bf16)
        nc.vector.memset(ones_col[:], 1.0)

        # ---- phase 0+1: x transpose + q/k/v projections ----
        # x goes through a bf16 DRAM bounce (SWDGE cast DMA), then one grouped
        # XBAR transpose per row-block writes all 16 d-strips of that t-range.
        # A t-range of xT carries ALL d, so v tiles / kT / qT chunks for early
        # t can start as soon as their block lands.
        with (
            tc.tile_pool(name="wpool", bufs=1) as wpool,
            tc.tile_pool(name="xpool", bufs=1) as xpool,
            tc.tile_pool(name="xstage", bufs=2) as xstage,
            tc.tile_pool(name="psA", bufs=4, space="PSUM") as psA,
        ):
            wq_bf = wpool.tile([128, ND, G * HD], bf16)
            wk_bf = wpool.tile([128, ND, HD], bf16)
            wv_bf = wpool.tile([128, ND, HD], bf16)
            xT = xpool.tile([128, ND, T], bf16)

            def _xblock(r0, r1):
                rs = slice(r0, r1)
                nc.gpsimd.dma_start(xbf[rs, :], xb[rs, :])
                nc.sync.dma_start_transpose(xT[:, :, rs], xbf[rs, :])

            def _xtile_staged(tt):
                # first tiles skip the DRAM bounce: fp32 load -> DVE cast ->
                # SBUF->SBUF XBAR transpose, so PE work starts ~15us earlier
                rs = slice(tt * 128, (tt + 1) * 128)
                xf = xstage.tile([128, D], f32, tag="xf")
                nc.sync.dma_start(xf[:], xb[rs, :])
                xc = xstage.tile([128, D], bf16, tag="xc")
                nc.vector.tensor_copy(xc[:], xf[:])
                return nc.sync.dma_start_transpose(xT[:, :, rs], xc[:])

            _xtile_staged(0)
            _xtile_staged(1)
            nc.gpsimd.dma_start(wv_bf[:], wv_s.rearrange("(dt p) h -> p dt h", p=128))
            nc.gpsimd.dma_start(wk_bf[:], wk_s.rearrange("(dt p) h -> p dt h", p=128))
            _xblock(256, 512)
            nc.gpsimd.dma_start(wq_bf[:], wq_s.rearrange("(dt p) h -> p dt h", p=128))
            _xblock(512, 1024)
            _xblock(1024, 1536)
            _xblock(1536, 2048)

            # projections, qc-major; v first within each qc (v tile kt needs
            # only one xT t-tile, so it is the earliest-ready PE work)
            for qc in range(T // 512):
                qs = slice(qc * 512, (qc + 1) * 512)
                for kt in range(4 * qc, 4 * qc + 4):
                    pv = psA.tile([128, 512], f32, tag="ps_proj", name="pv")
                    for dt in range(ND):
                        nc.tensor.matmul(
                            pv[:, :HD],
                            xT[:, dt, kt * 128 : (kt + 1) * 128],
                            wv_bf[:, dt, :],
                            start=(dt == 0), stop=(dt == ND - 1),
                        )
                    nc.scalar.copy(vB[:, kt, :], pv[:, :HD])
                pk = psA.tile([128, 512], f32, tag="ps_proj", name="pk")
                for dt in range(ND):
                    nc.tensor.matmul(
                        pk[:], wk_bf[:, dt, :], xT[:, dt, qs],
                        start=(dt == 0), stop=(dt == ND - 1),
                    )
                nc.scalar.copy(kT[:, qs], pk[:])
                for ht in range(G):
                    pq = psA.tile([128, 512], f32, tag="ps_proj", name="pq")
                    for dt in range(ND):
                        nc.tensor.matmul(
                            pq[:],
                            wq_bf[:, dt, ht * 128 : (ht + 1) * 128],
                            xT[:, dt, qs],
                            start=(dt == 0), stop=(dt == ND - 1),
                        )
                    nc.scalar.copy(qT[:, ht, qs], pq[:])

        # wo load: issued now so the DMA overlaps the attention phase
        nc.gpsimd.dma_start(wo_bf[:], wo_s.rearrange("(ht p) d -> p ht d", p=128))

        # ---- phase 2+3: attention per (half, head), then o-proj per half ----
        with (
            tc.tile_pool(name="apool", bufs=2) as apool,
            tc.tile_pool(name="opool", bufs=2) as opool,
            tc.tile_pool(name="ps_sT", bufs=2, space="PSUM") as ps_sT,
            tc.tile_pool(name="ps_av", bufs=1, space="PSUM") as ps_av,
            tc.tile_pool(name="ps_sum", bufs=1, space="PSUM") as ps_sum,
        ):
            for half in range(2):
                q0 = half * 1024
                for h in range(G):
                    attnT = apool.tile([128, NT, 1024], bf16, tag="attnT")
                    pav = ps_av.tile([128, 1024], f32, tag="av")
                    psums = [
                        ps_sum.tile([1, 512], f32, tag=f"sum{i}", name=f"psum{i}")
                        for i in range(2)
                    ]
                    def _av_sums(kt):
                        # v[kt] stays loaded across both qc, then ones_col
                        for qc in range(2):
                            nc.tensor.matmul(
                                pav[:, qc * 512 : (qc + 1) * 512],
                                vB[:, kt, :],
                                attnT[:, kt, qc * 512 : (qc + 1) * 512],
                                start=(kt == 0), stop=(kt == NT - 1),
                            )
                        for qc in range(2):
                            nc.tensor.matmul(
                                psums[qc][:],
                                ones_col[:],
                                attnT[:, kt, qc * 512 : (qc + 1) * 512],
                                start=(kt == 0), stop=(kt == NT - 1),
                            )

                    def _sums(kt):
                        for qc in range(2):
                            nc.tensor.matmul(
                                psums[qc][:],
                                ones_col[:],
                                attnT[:, kt, qc * 512 : (qc + 1) * 512],
                                start=(kt == 0), stop=(kt == NT - 1),
                            )

                    # av/sums lag exp by one kt so PE never waits on ACT
                    for kt in range(NT):
                        ks = slice(kt * 128, (kt + 1) * 128)
                        pst = ps_sT.tile([128, 1024], f32, tag="sT", name="pst")
                        for qc in range(2):
                            nc.tensor.matmul(
                                pst[:, qc * 512 : (qc + 1) * 512],
                                kT[:, ks],
                                qT[:, h, q0 + qc * 512 : q0 + (qc + 1) * 512],
                                start=True, stop=True,
                            )
                        nc.scalar.activation(
                            attnT[:, kt, :], pst[:], FT.Exp, scale=SCALE
                        )
                        if kt >= 1:
                            _av_sums(kt - 1)
                    # last kt: sums first so the normalization chain (which
                    # starts from the sums) unblocks as early as possible
                    _sums(NT - 1)
                    for qc in range(2):
                        nc.tensor.matmul(
                            pav[:, qc * 512 : (qc + 1) * 512],
                            vB[:, NT - 1, :],
                            attnT[:, NT - 1, qc * 512 : (qc + 1) * 512],
                            start=False, stop=True,
                        )
                    # normalization tail, ordered to un-block PE fast: evac the
                    # tiny sums (0.3us) -> PE broadcasts the SUMS via K=1 outer
                    # product -> full-width reciprocal + mul run off-path on DVE
                    bcs = []
                    for qc in range(2):
                        sum_sb = apool.tile([1, 512], f32, tag=f"sum_sb{qc}",
                                            name="sum_sb")
                        nc.vector.tensor_copy(sum_sb[:], psums[qc][:])
                        bc_in = apool.tile([128, 512], f32, tag=f"bcin{qc}",
                                           name="bc_in")
                        nc.gpsimd.partition_broadcast(bc_in[:], sum_sb[:])
                        bcs.append(bc_in)
                    av_sb = apool.tile([128, 1024], f32, tag="av_sb")
                    nc.vector.tensor_copy(av_sb[:], pav[:])
                    for qc in range(2):
                        bc_sb = apool.tile([128, 512], f32, tag=f"bc{qc}",
                                           name="bc_sb")
                        nc.vector.reciprocal(bc_sb[:], bcs[qc][:])
                        nc.vector.tensor_mul(
                            out=aoT[:, h, q0 + qc * 512 : q0 + (qc + 1) * 512],
                            in0=av_sb[:, qc * 512 : (qc + 1) * 512],
                            in1=bc_sb[:],
                        )

                # output projection for this half's 8 t-tiles
                for tt in range(half * 8, half * 8 + 8):
                    osb = opool.tile([128, D], f32, tag="osb")
                    for dcp in range(2):
                        po = ps_sT.tile([128, 1024], f32, tag="sT", name="po")
                        for ht in range(G):
                            # both 512-chunks share one loaded aoT tile
                            for j in range(2):
                                dc = dcp * 2 + j
                                nc.tensor.matmul(
                                    po[:, j * 512 : (j + 1) * 512],
                                    aoT[:, ht, tt * 128 : (tt + 1) * 128],
                                    wo_bf[:, ht, dc * 512 : (dc + 1) * 512],
                                    start=(ht == 0), stop=(ht == G - 1),
                                )
                        nc.vector.tensor_copy(
                            osb[:, dcp * 1024 : (dcp + 1) * 1024], po[:]
                        )
                    nc.sync.dma_start(out_p[tt * 128 : (tt + 1) * 128, :], osb[:])

    nc.finalize()
    return nc


def _get_nc():
    if "nc" not in _CACHE:
        _CACHE["nc"] = _build_nc()
    return _CACHE["nc"]


def _shard_inputs(x, wq, wk, wv, wo):
    in_maps = []
    for c in range(NCORES):
        b, g = divmod(c, 4)
        in_maps.append(
            {
                "xb": np.ascontiguousarray(x[b]),
                "wq_s": np.ascontiguousarray(wq[:, g * G * HD : (g + 1) * G * HD]),
                "wk_s": np.ascontiguousarray(wk[:, g * HD : (g + 1) * HD]),
                "wv_s": np.ascontiguousarray(wv[:, g * HD : (g + 1) * HD]),
                "wo_s": np.ascontiguousarray(wo[g * G * HD : (g + 1) * G * HD, :]),
            }
        )
    return in_maps


def kernel(x, wq, wk, wv, wo, _trace=False, _trace_kwargs=None):
    from concourse.bass_utils import run_bass_kernel_spmd

    x = np.asarray(x, dtype=np.float32)
    wq = np.asarray(wq, dtype=np.float32)
    wk = np.asarray(wk, dtype=np.float32)
    wv = np.asarray(wv, dtype=np.float32)
    wo = np.asarray(wo, dtype=np.float32)

    nc = _get_nc()
    in_maps = _shard_inputs(x, wq, wk, wv, wo)
    res = run_bass_kernel_spmd(
        nc, in_maps, list(range(NCORES)), trace=_trace, **(_trace_kwargs or {})
    )
    out = np.zeros((B, T, D), np.float32)
    for c in range(NCORES):
        out[c // 4] += res.results[c]["out_p"]
    if _trace:
        _CACHE["last_results"] = res
    return out


# revision 34
# speedup vs baseline: 20823.8659x; 1.0195x over previous
"""GQA attention layer (B=2, T=2048, D=2048, H=16, HKV=4, HD=128) on 8 NeuronCores.

Sharding: 8 cores = 2 batches x 4 head-groups. Each group of 4 consecutive Q
heads shares exactly one KV head (GQA rep=4), so core c handles batch c//4 and
q-heads [4*(c%4), 4*(c%4)+4) with kv-head c%4. Each core computes a partial
output projection (its 4 heads' slice of wo); the host sums the 4 partials per
batch.

On-core layout (bf16 matmul inputs, fp32 PSUM accumulation):
  xT   [d, t]   via fp32->bf16 cast DMA to HBM scratch + grouped XBAR
                transpose DMA per row-block (first two 128-row tiles go
                through SBUF staging instead, to start PE work early)
  qT   [hd, t]  = matmul(lhsT=wq[d,hd], rhs=xT[d,t])
  kT   [hd, t]  = matmul(lhsT=wk[d,hd], rhs=xT[d,t])
  v    [t, hd]  = matmul(lhsT=xT[d,t], rhs=wv[d,hd])
  sT   [key, q] = matmul(lhsT=kT[:,keytile], rhs=qT[:,qchunk])   (scores^T)
  attnT[key, q] = Exp(sT / sqrt(HD))             (ACT; no max-subtraction --
                                                  |scores|<~6 so exp is safe)
  avT  [hd, q]  = sum_kt matmul(lhsT=v[kt], rhs=attnT[kt])       (unnormalized)
  sums [1, q]   = sum_kt matmul(lhsT=ones_col, rhs=attnT[kt])    (softmax denom)
  aoT  [hd, q]  = avT * (1/gpsimd_partition_broadcast(sums))     (DVE mult)
  out  [t, d]   = sum_ht matmul(lhsT=aoT[:,ttile], rhs=wo[hd,d])

av/sums matmuls lag the exp by one key-tile so PE never stalls on ACT, and
the normalization chain starts from a cheap sums evacuation so the PSUM
accumulators recycle quickly at head boundaries.

Queries are processed in two halves; the output projection for a half runs
interleaved with the next half's attention (same PSUM slots as scoresT).
"""

import math

import numpy as np

B, T, D = 2, 2048, 2048
H, HKV, HD = 16, 4, 128
G = 4  # q-heads per core
NCORES = 8
ND = D // 128  # 16 d-chunks
NT = T // 128  # 16 t-tiles

_CACHE = {}


def _build_nc():
    from contextlib import ExitStack

    import concourse.bacc as bacc
    import concourse.mybir as mybir
    import concourse.tile as tile

    f32, bf16 = mybir.dt.float32, mybir.dt.bfloat16
    FT = mybir.ActivationFunctionType
    SCALE = 1.0 / math.sqrt(HD)

    nc = bacc.Bacc("TRN2", target_bir_lowering=False, debug=False, num_devices=NCORES)
    xb = nc.declare_dram_parameter("xb", [T, D], f32, isOutput=False)
    wq_s = nc.declare_dram_parameter("wq_s", [D, G * HD], f32, isOutput=False)
    wk_s = nc.declare_dram_parameter("wk_s", [D, HD], f32, isOutput=False)
    wv_s = nc.declare_dram_parameter("wv_s", [D, HD], f32, isOutput=False)
    wo_s = nc.declare_dram_parameter("wo_s", [G * HD, D], f32, isOutput=False)
    out_p = nc.declare_dram_parameter("out_p", [T, D], bf16, isOutput=True)

    with tile.TileContext(nc) as tc, ExitStack() as ctx:
        dram = ctx.enter_context(tc.tile_pool(name="dram", bufs=1, space="DRAM"))
        persist = ctx.enter_context(tc.tile_pool(name="persist", bufs=1))

        xbf = dram.tile([T, D], bf16)

        qT = persist.tile([128, G, T], bf16)
        kT = persist.tile([128, T], bf16)
        vB = persist.tile([128, NT, HD], bf16)
        aoT = persist.tile([128, G, T], bf16)
        wo_bf = persist.tile([128, G, D], bf16)
        ones_col = persist.tile([128, 1], bf16)
        nc.vector.memset(ones_col[:], 1.0)

        # ---- phase 0+1: x transpose + q/k/v projections ----
        # x goes through a bf16 DRAM bounce (SWDGE cast DMA), then one grouped
        # XBAR transpose per row-block writes all 16 d-strips of that t-range.
        # A t-range of xT carries ALL d, so v tiles / kT / qT chunks for early
        # t can start as soon as their block lands.
        with (
            tc.tile_pool(name="wpool", bufs=1) as wpool,
            tc.tile_pool(name="xpool", bufs=1) as xpool,
            tc.tile_pool(name="xstage", bufs=2) as xstage,
            tc.tile_pool(name="psA", bufs=4, space="PSUM") as psA,
        ):
            wq_bf = wpool.tile([128, ND, G * HD], bf16)
            wk_bf = wpool.tile([128, ND, HD], bf16)
            wv_bf = wpool.tile([128, ND, HD], bf16)
            xT = xpool.tile([128, ND, T], bf16)

            def _xblock(r0, r1):
                rs = slice(r0, r1)
                nc.gpsimd.dma_start(xbf[rs, :], xb[rs, :])
                nc.sync.dma_start_transpose(xT[:, :, rs], xbf[rs, :])

            def _xtile_staged(tt):
                # first tiles skip the DRAM bounce: fp32 load -> DVE cast ->
                # SBUF->SBUF XBAR transpose, so PE work starts ~15us earlier
                rs = slice(tt * 128, (tt + 1) * 128)
                xf = xstage.tile([128, D], f32, tag="xf")
                nc.sync.dma_start(xf[:], xb[rs, :])
                xc = xstage.tile([128, D], bf16, tag="xc")
                nc.vector.tensor_copy(xc[:], xf[:])
                return nc.sync.dma_start_transpose(xT[:, :, rs], xc[:])

            _xtile_staged(0)
            _xtile_staged(1)
            nc.gpsimd.dma_start(wv_bf[:], wv_s.rearrange("(dt p) h -> p dt h", p=128))
            nc.gpsimd.dma_start(wk_bf[:], wk_s.rearrange("(dt p) h -> p dt h", p=128))
            _xblock(256, 512)
            nc.gpsimd.dma_start(wq_bf[:], wq_s.rearrange("(dt p) h -> p dt h", p=128))
            _xblock(512, 1024)
            _xblock(1024, 1536)
            _xblock(1536, 2048)

            # projections, qc-major; v first within each qc (v tile kt needs
            # only one xT t-tile, so it is the earliest-ready PE work)
            for qc in range(T // 512):
                qs = slice(qc * 512, (qc + 1) * 512)
                for kt in range(4 * qc, 4 * qc + 4):
                    pv = psA.tile([128, 512], f32, tag="ps_proj", name="pv")
                    for dt in range(ND):
                        nc.tensor.matmul(
                            pv[:, :HD],
                            xT[:, dt, kt * 128 : (kt + 1) * 128],
                            wv_bf[:, dt, :],
                            start=(dt == 0), stop=(dt == ND - 1),
                        )
                    nc.scalar.copy(vB[:, kt, :], pv[:, :HD])
                pk = psA.tile([128, 512], f32, tag="ps_proj", name="pk")
                for dt in range(ND):
                    nc.tensor.matmul(
                        pk[:], wk_bf[:, dt, :], xT[:, dt, qs],
                        start=(dt == 0), stop=(dt == ND - 1),
                    )
                nc.scalar.copy(kT[:, qs], pk[:])
                for ht in range(G):
                    pq = psA.tile([128, 512], f32, tag="ps_proj", name="pq")
                    for dt in range(ND):
                        nc.tensor.matmul(
                            pq[:],
                            wq_bf[:, dt, ht * 128 : (ht + 1) * 128],
                            xT[:, dt, qs],
                            start=(dt == 0), stop=(dt == ND - 1),
                        )
                    nc.scalar.copy(qT[:, ht, qs], pq[:])

        # wo load: issued now so the DMA overlaps the attention phase
        nc.gpsimd.dma_start(wo_bf[:], wo_s.rearrange("(ht p) d -> p ht d", p=128))

        # ---- phase 2+3: attention per (half, head), then o-proj per half ----
        with (
            tc.tile_pool(name="apool", bufs=2) as apool,
            tc.tile_pool(name="opool", bufs=2) as opool,
            tc.tile_pool(name="ps_sT", bufs=2, space="PSUM") as ps_sT,
            tc.tile_pool(name="ps_av", bufs=1, space="PSUM") as ps_av,
            tc.tile_pool(name="ps_sum", bufs=1, space="PSUM") as ps_sum,
        ):
            for half in range(2):
                q0 = half * 1024
                for h in range(G):
                    attnT = apool.tile([128, NT, 1024], bf16, tag="attnT")
                    pav = ps_av.tile([128, 1024], f32, tag="av")
                    psums = [
                        ps_sum.tile([1, 512], f32, tag=f"sum{i}", name=f"psum{i}")
                        for i in range(2)
                    ]
                    def _av_sums(kt):
                        # v[kt] stays loaded across both qc, then ones_col
                        for qc in range(2):
                            nc.tensor.matmul(
                                pav[:, qc * 512 : (qc + 1) * 512],
                                vB[:, kt, :],
                                attnT[:, kt, qc * 512 : (qc + 1) * 512],
                                start=(kt == 0), stop=(kt == NT - 1),
                            )
                        for qc in range(2):
                            nc.tensor.matmul(
                                psums[qc][:],
                                ones_col[:],
                                attnT[:, kt, qc * 512 : (qc + 1) * 512],
                                start=(kt == 0), stop=(kt == NT - 1),
                            )

                    def _sums(kt):
                        for qc in range(2):
                            nc.tensor.matmul(
                                psums[qc][:],
                                ones_col[:],
                                attnT[:, kt, qc * 512 : (qc + 1) * 512],
                                start=(kt == 0), stop=(kt == NT - 1),
                            )

                    # av/sums lag exp by one kt so PE never waits on ACT
                    for kt in range(NT):
                        ks = slice(kt * 128, (kt + 1) * 128)
                        pst = ps_sT.tile([128, 1024], f32, tag="sT", name="pst")
                        for qc in range(2):
                            nc.tensor.matmul(
                                pst[:, qc * 512 : (qc + 1) * 512],
                                kT[:, ks],
                                qT[:, h, q0 + qc * 512 : q0 + (qc + 1) * 512],
                                start=True, stop=True,
                            )
                        nc.scalar.activation(
                            attnT[:, kt, :], pst[:], FT.Exp, scale=SCALE
                        )
                        if kt >= 1:
                            _av_sums(kt - 1)
                    # last kt: sums first so the normalization chain (which
                    # starts from the sums) unblocks as early as possible
                    _sums(NT - 1)
                    for qc in range(2):
                        nc.tensor.matmul(
                            pav[:, qc * 512 : (qc + 1) * 512],
                            vB[:, NT - 1, :],
                            attnT[:, NT - 1, qc * 512 : (qc + 1) * 512],
                            start=False, stop=True,
                        )
                    # normalization tail, ordered to un-block PE fast: evac the
                    # tiny sums (0.3us) -> PE broadcasts the SUMS via K=1 outer
                    # product -> full-width reciprocal + mul run off-path on DVE
                    bcs = []
                    for qc in range(2):
                        sum_sb = apool.tile([1, 512], f32, tag=f"sum_sb{qc}",
                                            name="sum_sb")
                        nc.vector.tensor_copy(sum_sb[:], psums[qc][:])
                        bc_in = apool.tile([128, 512], f32, tag=f"bcin{qc}",
                                           name="bc_in")
                        nc.gpsimd.partition_broadcast(bc_in[:], sum_sb[:])
                        bcs.append(bc_in)
                    av_sb = apool.tile([128, 1024], f32, tag="av_sb")
                    nc.vector.tensor_copy(av_sb[:], pav[:])
                    for qc in range(2):
                        bc_sb = apool.tile([128, 512], f32, tag=f"bc{qc}",
                                           name="bc_sb")
                        nc.vector.reciprocal(bc_sb[:], bcs[qc][:])
                        nc.vector.tensor_mul(
                            out=aoT[:, h, q0 + qc * 512 : q0 + (qc + 1) * 512],
                            in0=av_sb[:, qc * 512 : (qc + 1) * 512],
                            in1=bc_sb[:],
                        )

                # output projection for this half's 8 t-tiles
                for tt in range(half * 8, half * 8 + 8):
                    osb = opool.tile([128, D], bf16, tag="osb")
                    for dcp in range(2):
                        po = ps_sT.tile([128, 1024], f32, tag="sT", name="po")
                        for ht in range(G):
                            # both 512-chunks share one loaded aoT tile
                            for j in range(2):
                                dc = dcp * 2 + j
                                nc.tensor.matmul(
                                    po[:, j * 512 : (j + 1) * 512],
                                    aoT[:, ht, tt * 128 : (tt + 1) * 128],
                                    wo_bf[:, ht, dc * 512 : (dc + 1) * 512],
                                    start=(ht == 0), stop=(ht == G - 1),
                                )
                        nc.vector.tensor_copy(
                            osb[:, dcp * 1024 : (dcp + 1) * 1024], po[:]
                        )
                    nc.sync.dma_start(out_p[tt * 128 : (tt + 1) * 128, :], osb[:])

    nc.finalize()
    return nc


def _get_nc():
    if "nc" not in _CACHE:
        _CACHE["nc"] = _build_nc()
    return _CACHE["nc"]


def _shard_inputs(x, wq, wk, wv, wo):
    in_maps = []
    for c in range(NCORES):
        b, g = divmod(c, 4)
        in_maps.append(
            {
                "xb": np.ascontiguousarray(x[b]),
                "wq_s": np.ascontiguousarray(wq[:, g * G * HD : (g + 1) * G * HD]),
                "wk_s": np.ascontiguousarray(wk[:, g * HD : (g + 1) * HD]),
                "wv_s": np.ascontiguousarray(wv[:, g * HD : (g + 1) * HD]),
                "wo_s": np.ascontiguousarray(wo[g * G * HD : (g + 1) * G * HD, :]),
            }
        )
    return in_maps


def kernel(x, wq, wk, wv, wo, _trace=False, _trace_kwargs=None):
    from concourse.bass_utils import run_bass_kernel_spmd

    x = np.asarray(x, dtype=np.float32)
    wq = np.asarray(wq, dtype=np.float32)
    wk = np.asarray(wk, dtype=np.float32)
    wv = np.asarray(wv, dtype=np.float32)
    wo = np.asarray(wo, dtype=np.float32)

    nc = _get_nc()
    in_maps = _shard_inputs(x, wq, wk, wv, wo)
    res = run_bass_kernel_spmd(
        nc, in_maps, list(range(NCORES)), trace=_trace, **(_trace_kwargs or {})
    )
    out = np.zeros((B, T, D), np.float32)
    for c in range(NCORES):
        out[c // 4] += res.results[c]["out_p"].astype(np.float32)
    if _trace:
        _CACHE["last_results"] = res
    return out
